# revision 7
# baseline (speedup 1.0000x reference)
"""Trainium2 Bass kernel for nn_DecoderStack (self-attn + cross-attn +
2-layer GELU FFN, shared decoder LN), 8-core data-parallel.

Sharding: 8 cores = 4 batches x 2 query-halves. Core c handles batch b=c//2,
query half h=c%2 (1024 tokens). K/V are computed per-core over the full
2048-token context (no collectives).

Layout: activations feature-major [D, S] on chip (D on partitions), weights
consumed as-is [d_in, d_out]. All matmuls in float32r (TF32-like, full PE
rate). Scores computed transposed [t, s]; softmax denominator via ones-column
matmuls (partition-dim sums); row broadcasts via GpSimd partition_broadcast.
K and V spill to DRAM and restream (SBUF budget).
"""
import sys
for _p in ("/opt/trn_rl_repo", "/root/.axon_site/_ro/trn_rl_repo"):
    if _p not in sys.path:
        sys.path.append(_p)

import numpy as np

import concourse.bass as bass
import concourse.tile as tile
from concourse import bacc, mybir
from concourse.bass_utils import run_bass_kernel_spmd

f32 = mybir.dt.float32
f32r = mybir.dt.float32r
AF = mybir.ActivationFunctionType
ALU = mybir.AluOpType

N_CORES = 8
B, S, T, D = 4, 2048, 2048, 1024
SH = S // 2          # per-core query tokens
KD = D // 128        # 8 d-tiles
TM = T // 128        # 16 t-tiles
SCALE = 1.0 / 8.0
LN_EPS = 1e-5
LN_RD = 1.0 / D

COLS = ["bq1", "bk1", "bq2", "bk2", "gm1", "bm1", "gm2", "bm2",
        "gd", "bd", "fb0", "fb1"]
NCOL = len(COLS)
ONES_COL = NCOL * 8  # last column of the packed cols input


def build_decoder(nc, taps=False, reps=0):
    """Emit the full per-core decoder program. Returns tap tensor names."""
    def din(name, shape, dt=f32r):
        return nc.dram_tensor(name, shape, dt, kind="ExternalInput").ap()

    xq = din("xq", [KD, 128, SH])          # x[b,half].T feature-major
    xkv = din("xkv", [KD, 128, T])         # x[b].T full
    ykv = din("ykv", [KD, 128, T])         # y[b].T full
    w = {n: din("w_" + n, [D, D]) for n in
         ["q1", "k1", "v1", "q2", "k2", "v2", "f0", "f1"]}
    cols_in = din("cols", [128, NCOL * 8 + 1], f32r)  # + ones column
    bv1_r = din("bv1", [1, D], f32)
    bv2_r = din("bv2", [1, D], f32)
    out = nc.dram_tensor("out", [KD, 128, SH], f32, kind="ExternalOutput").ap()

    tap_names = []

    with tile.TileContext(nc, pool_alloc_mode="queue") as tc:
        import contextlib
        rep_ctx = tc.For_i(0, reps, 1) if reps else contextlib.nullcontext()
        es = []

        def open_pool(name, bufs=1, space="SBUF"):
            cm = tc.tile_pool(name=name, bufs=bufs, space=space)
            pool = cm.__enter__()
            es.append(cm)
            return pool

        rep_ctx.__enter__()
        p_w = open_pool("w", bufs=2)          # weight halves [128,8,512] 16K
        p_st4 = open_pool("st4", bufs=2)      # [128,8,128] k/v streams 4K
        p_stage = open_pool("stage", bufs=2)  # [128,1024] staging 4K
        p_bc = open_pool("bc", bufs=2)        # [128,1024] broadcasts 4K
        p_rows = open_pool("rows", bufs=3)    # [1,1024] rows 4K
        p_cmn = open_pool("cmn", bufs=1)      # cols 4K
        p_dram = open_pool("dram", bufs=1, space="DRAM")
        p_psm = open_pool("psm", bufs=4, space="PSUM")   # [128,512]
        p_psr = open_pool("psr", bufs=4, space="PSUM")   # [1,512]

        cols_sb = p_cmn.tile([128, NCOL * 8 + 1], f32r, name="cols_sb")
        nc.sync.dma_start(cols_sb[:], cols_in)
        ones_sb = cols_sb[:, ONES_COL:ONES_COL + 1]      # f32r ones column

        def col(name, j):
            c = COLS.index(name)
            return cols_sb[:, c * 8 + j: c * 8 + j + 1].bitcast(f32)

        def tap(name, ap_src, shape, dt_src=f32r):
            if not taps:
                return
            t = nc.dram_tensor("tap_" + name, shape, f32,
                               kind="ExternalOutput").ap()
            tap_names.append("tap_" + name)
            nc.sync.dma_start(t, ap_src.bitcast(f32) if dt_src == f32r else ap_src)

        def load_w_halves(wap):
            """Weight [D, D] as two halves [128, 8, 512] (d_out split)."""
            wr = wap.rearrange("(ko kp) d -> kp ko d", kp=128)
            halves = []
            for hf in range(2):
                t = p_w.tile([128, KD, 512], f32r, tag="w", name=f"wh{hf}")
                for k in range(KD):
                    nc.sync.dma_start(t[:, k, :], wr[:, k, hf * 512:(hf + 1) * 512])
                halves.append(t)
            return halves

        def proj(out_write, wap, rhs_src, n_tok):
            """Feature-major projection: psum[m-tile, 512chunk] = w.T @ rhs.

            out_write(m, tch, ps): epilogue for the [128,512] PSUM tile.
            rhs_src: DRAM AP [KD, 128, n_tok] or SBUF tile [128, KD, n_tok].
            """
            wh = load_w_halves(wap)
            nch = n_tok // 512
            from_dram = rhs_src.space == bass.MemorySpace.DRAM
            pk_cm = tc.tile_pool(name="kvch", bufs=2)
            pk = pk_cm.__enter__()
            for tch in range(nch):
                sl = slice(tch * 512, (tch + 1) * 512)
                if from_dram:
                    kvc = pk.tile([128, KD, 512], f32r, tag="kv", name="kvc")
                    for k in range(KD):
                        nc.sync.dma_start(kvc[:, k, :], rhs_src[k, :, sl])
                    rhs = lambda k: kvc[:, k, :]
                else:
                    rhs = lambda k, sl=sl: rhs_src[:, k, sl]
                for m in range(KD):
                    ps = p_psm.tile([128, 512], f32, tag="mm", name="proj_ps")
                    whf = wh[m // 4]
                    ml = m % 4
                    for k in range(KD):
                        nc.tensor.matmul(
                            ps[:], lhsT=whf[:, k, ml * 128:(ml + 1) * 128],
                            rhs=rhs(k), start=(k == 0), stop=(k == KD - 1))
                    out_write(m, tch, ps)
            pk_cm.__exit__(None, None, None)

        def layernorm(z_sb, g_cb, b_cb, out_cb):
            """LN over the feature dim (128 partitions x KD) of [128,KD,SH]."""
            ps_s = [p_psr.tile([1, 512], f32, tag="row", name=f"lns{i}")
                    for i in range(2)]
            ps_q = [p_psr.tile([1, 512], f32, tag="row", name=f"lnq{i}")
                    for i in range(2)]
            for m in range(KD):
                sq = p_stage.tile([128, 1024], f32r, tag="stage", name="lnsq")
                nc.scalar.activation(sq[:], z_sb[:, m, :], AF.Square)
                for sch in range(2):
                    sl = slice(sch * 512, (sch + 1) * 512)
                    nc.tensor.matmul(ps_s[sch][:], lhsT=ones_sb,
                                     rhs=z_sb[:, m, sl],
                                     start=(m == 0), stop=(m == KD - 1))
                    nc.tensor.matmul(ps_q[sch][:], lhsT=ones_sb,
                                     rhs=sq[:, sl],
                                     start=(m == 0), stop=(m == KD - 1))
            sum_r = p_rows.tile([1, 1024], f32, tag="row", name="sum_r")
            sq_r = p_rows.tile([1, 1024], f32, tag="row", name="sq_r")
            for sch in range(2):
                sl = slice(sch * 512, (sch + 1) * 512)
                nc.scalar.copy(sum_r[:, sl], ps_s[sch][:])
                nc.scalar.copy(sq_r[:, sl], ps_q[sch][:])
            mean = p_rows.tile([1, 1024], f32, tag="mean", bufs=1, name="mean")
            nc.vector.tensor_scalar(mean[:], sum_r[:], LN_RD, None, op0=ALU.mult)
            vp = p_rows.tile([1, 1024], f32, tag="row", name="vp")
            nc.vector.tensor_scalar(vp[:], sq_r[:], LN_RD, LN_EPS,
                                    op0=ALU.mult, op1=ALU.add)
            msq = p_rows.tile([1, 1024], f32, tag="row", name="msq")
            nc.vector.tensor_mul(msq[:], mean[:], mean[:])
            varc = p_rows.tile([1, 1024], f32, tag="row", name="varc")
            nc.vector.tensor_sub(varc[:], vp[:], msq[:])
            std = p_rows.tile([1, 1024], f32, tag="row", name="std")
            nc.scalar.activation(std[:], varc[:], AF.Sqrt)
            rstd = p_rows.tile([1, 1024], f32, tag="row", name="rstd")
            nc.vector.reciprocal(rstd[:], std[:])
            cr = p_rows.tile([1, 1024], f32, tag="row", name="cr")
            nc.vector.tensor_mul(cr[:], mean[:], rstd[:])
            rstd_bc = p_bc.tile([128, 1024], f32, tag="bc", name="rstd_bc")
            nc.gpsimd.partition_broadcast(rstd_bc[:], rstd[:])
            c_bc = p_bc.tile([128, 1024], f32, tag="bc", name="c_bc")
            nc.gpsimd.partition_broadcast(c_bc[:], cr[:])
            for m in range(KD):
                t1 = p_stage.tile([128, 1024], f32, tag="stage", name="ln_t1")
                nc.vector.tensor_mul(t1[:], z_sb[:, m, :].bitcast(f32), rstd_bc[:])
                t2 = p_stage.tile([128, 1024], f32, tag="stage", name="ln_t2")
                nc.vector.tensor_sub(t2[:], t1[:], c_bc[:])
                nc.vector.tensor_scalar(out_cb(m), t2[:], g_cb(m), b_cb(m),
                                        op0=ALU.mult, op1=ALU.add)

        def attention_block(qin_d, kv_d, wq, wk, wv, bqn, bkn, bv_row,
                            gmn, bmn, xout_d, blk):
            # ---- Q projection (SBUF resident) ----
            pq_cm = tc.tile_pool(name=f"q{blk}", bufs=1)
            pq = pq_cm.__enter__()
            q_sb = pq.tile([128, KD, SH], f32r, name="q_sb")

            def qwrite(m, sch, ps):
                nc.vector.tensor_scalar(q_sb[:, m, sch * 512:(sch + 1) * 512],
                                        ps[:], col(bqn, m), None, op0=ALU.add)
            proj(qwrite, wq, qin_d, SH)
            tap(f"Q{blk}", q_sb[:], [128, KD, SH])

            # ---- K projection -> DRAM spill [KD, 128, T] ----
            k_spill = p_dram.tile([KD, 128, T], f32r, tag="kspill",
                                  name="k_spill")

            def kwrite(m, tch, ps):
                st = p_stage.tile([128, 1024], f32r, tag="stage", name="k_st")
                nc.vector.tensor_scalar(st[:, 0:512], ps[:], col(bkn, m), None,
                                        op0=ALU.add)
                nc.sync.dma_start(k_spill[m, :, tch * 512:(tch + 1) * 512],
                                  st[:, 0:512])
            proj(kwrite, wk, kv_d, T)

            # ---- V projection -> DRAM spill [TM, 128, D] (token-major) ----
            bv_sb = p_rows.tile([1, D], f32, tag="row", name="bv_sb")
            nc.sync.dma_start(bv_sb[:], bv_row)
            bv_bc = p_bc.tile([128, D], f32, tag="bc", name="bv_bc")
            nc.gpsimd.partition_broadcast(bv_bc[:], bv_sb[:])
            v_spill = p_dram.tile([TM, 128, D], f32r, tag="vspill",
                                  name="v_spill")
            whv = load_w_halves(wv)
            for tm in range(TM):
                kvt = p_st4.tile([128, KD, 128], f32r, tag="st4", name="kvt")
                nc.sync.dma_start(
                    kvt[:], kv_d[:, :, tm * 128:(tm + 1) * 128]
                    .rearrange("ko p t -> p ko t"))
                for dch in range(2):
                    ps = p_psm.tile([128, 512], f32, tag="mm", name="v_ps")
                    for k in range(KD):
                        nc.tensor.matmul(
                            ps[:], lhsT=kvt[:, k, :], rhs=whv[dch][:, k, :],
                            start=(k == 0), stop=(k == KD - 1))
                    st = p_stage.tile([128, 1024], f32r, tag="stage",
                                      name="v_st")
                    nc.vector.tensor_add(
                        st[:, 0:512], ps[:],
                        bv_bc[:, dch * 512:(dch + 1) * 512])
                    nc.sync.dma_start(
                        v_spill[tm, :, dch * 512:(dch + 1) * 512], st[:, 0:512])

            # ---- scores.T -> exp -> E ; denominator ----
            pe_cm = tc.tile_pool(name=f"e{blk}", bufs=1)
            pe = pe_cm.__enter__()
            e_sb = pe.tile([128, TM, SH], f32r, name="e_sb")
            ps_d = [p_psr.tile([1, 512], f32, tag="row", name=f"dn{i}")
                    for i in range(2)]
            for tm in range(TM):
                kt = p_st4.tile([128, KD, 128], f32r, tag="st4", name="kt")
                nc.sync.dma_start(
                    kt[:], k_spill[:, :, tm * 128:(tm + 1) * 128]
                    .rearrange("ko p t -> p ko t"))
                for sch in range(2):
                    sl = slice(sch * 512, (sch + 1) * 512)
                    ps = p_psm.tile([128, 512], f32, tag="mm", name="sc_ps")
                    for k in range(KD):
                        nc.tensor.matmul(ps[:], lhsT=kt[:, k, :],
                                         rhs=q_sb[:, k, sl],
                                         start=(k == 0), stop=(k == KD - 1))
                    nc.scalar.activation(e_sb[:, tm, sl], ps[:], AF.Exp,
                                         scale=SCALE)
                    nc.tensor.matmul(ps_d[sch][:], lhsT=ones_sb,
                                     rhs=e_sb[:, tm, sl],
                                     start=(tm == 0), stop=(tm == TM - 1))
            den_r = p_rows.tile([1, 1024], f32, tag="row", name="den_r")
            for sch in range(2):
                nc.scalar.copy(den_r[:, sch * 512:(sch + 1) * 512], ps_d[sch][:])
            tap(f"den{blk}", den_r[:], [1, 1024], f32)
            rden = p_rows.tile([1, 1024], f32, tag="row", name="rden")
            nc.vector.reciprocal(rden[:], den_r[:])
            rden_bc = p_bc.tile([128, 1024], f32, tag="bc", name="rden_bc")
            nc.gpsimd.partition_broadcast(rden_bc[:], rden[:])

            # ---- PV -> normalize -> +resid -> Z (in-place chain) ----
            pz_cm = tc.tile_pool(name=f"z{blk}", bufs=1)
            pz = pz_cm.__enter__()
            z_sb = pz.tile([128, KD, SH], f32r, name="z_sb")
            for m in range(KD):
                vh = []
                for hfm in range(2):
                    vt = p_st4.tile([128, 8, 128], f32r, tag="st4", name="vh")
                    nc.sync.dma_start(
                        vt[:], v_spill[hfm * 8:(hfm + 1) * 8, :,
                                       m * 128:(m + 1) * 128]
                        .rearrange("tm p d -> p tm d"))
                    vh.append(vt)
                psu = [p_psm.tile([128, 512], f32, tag="mm", name=f"pv{i}")
                       for i in range(2)]
                for tm in range(TM):
                    vt = vh[tm // 8][:, tm % 8, :]
                    for sch in range(2):
                        sl = slice(sch * 512, (sch + 1) * 512)
                        nc.tensor.matmul(psu[sch][:], lhsT=vt,
                                         rhs=e_sb[:, tm, sl],
                                         start=(tm == 0), stop=(tm == TM - 1))
                for sch in range(2):
                    sl = slice(sch * 512, (sch + 1) * 512)
                    rt = p_stage.tile([128, 1024], f32r, tag="stage",
                                      name="res_t")
                    nc.sync.dma_start(rt[:, 0:512], qin_d[m, :, sl])
                    t1 = p_stage.tile([128, 1024], f32, tag="stage",
                                      name="pv_t1")
                    nc.vector.tensor_mul(t1[:, 0:512], psu[sch][:],
                                         rden_bc[:, sl])
                    nc.vector.tensor_add(z_sb[:, m, sl], t1[:, 0:512],
                                         rt[:, 0:512].bitcast(f32))
            tap(f"Z1_{blk}", z_sb[:], [128, KD, SH])

            # ---- LN_m (in-place) ; + resid (in-place) ; LN_d -> xout ----
            layernorm(z_sb, lambda m: col(gmn, m), lambda m: col(bmn, m),
                      lambda m: z_sb[:, m, :])
            for m in range(KD):
                for sch in range(2):
                    sl = slice(sch * 512, (sch + 1) * 512)
                    rt = p_stage.tile([128, 1024], f32r, tag="stage",
                                      name="res2_t")
                    nc.sync.dma_start(rt[:, 0:512], qin_d[m, :, sl])
                    nc.vector.tensor_add(z_sb[:, m, sl], z_sb[:, m, sl],
                                         rt[:, 0:512])
            sts = {}

            def xcb(m):
                st = p_stage.tile([128, 1024], f32r, tag="stage", name="xo_st")
                sts[m] = st
                return st[:, 0:SH]
            layernorm(z_sb, lambda m: col("gd", m), lambda m: col("bd", m), xcb)
            for m in range(KD):
                nc.sync.dma_start(xout_d[m, :, :], sts[m][:, 0:SH])
            pz_cm.__exit__(None, None, None)
            pe_cm.__exit__(None, None, None)
            pq_cm.__exit__(None, None, None)

        # ================= decoder =================
        x1_d = p_dram.tile([KD, 128, SH], f32r, tag="x1", name="x1_d")
        attention_block(xq, xkv, w["q1"], w["k1"], w["v1"], "bq1", "bk1",
                        bv1_r, "gm1", "bm1", x1_d, 1)
        x2_d = p_dram.tile([KD, 128, SH], f32r, tag="x2", name="x2_d")
        attention_block(x1_d, ykv, w["q2"], w["k2"], w["v2"], "bq2", "bk2",
                        bv2_r, "gm2", "bm2", x2_d, 2)

        # ================= FFN =================
        ph_cm = tc.tile_pool(name="h1p", bufs=1)
        ph = ph_cm.__enter__()
        h1 = ph.tile([128, KD, SH], f32r, name="h1")

        def h1w(m, sch, ps):
            nc.scalar.activation(h1[:, m, sch * 512:(sch + 1) * 512], ps[:],
                                 AF.Gelu, bias=col("fb0", m))
        proj(h1w, w["f0"], x2_d, SH)

        pz5_cm = tc.tile_pool(name="z5p", bufs=1)
        pz5 = pz5_cm.__enter__()
        z5 = pz5.tile([128, KD, SH], f32r, name="z5")

        def h2w(m, sch, ps):
            sl = slice(sch * 512, (sch + 1) * 512)
            t1 = p_stage.tile([128, 1024], f32, tag="stage", name="h2_t")
            nc.scalar.activation(t1[:, 0:512], ps[:], AF.Gelu,
                                 bias=col("fb1", m))
            rt = p_stage.tile([128, 1024], f32r, tag="stage", name="resf_t")
            nc.sync.dma_start(rt[:, 0:512], x2_d[m, :, sl])
            nc.vector.tensor_add(z5[:, m, sl], t1[:, 0:512],
                                 rt[:, 0:512].bitcast(f32))
        proj(h2w, w["f1"], h1, SH)

        outs = {}

        def out_cb(m):
            st = p_stage.tile([128, 1024], f32r, tag="stage", name="out_st")
            outs[m] = st
            return st[:, 0:SH]
        layernorm(z5, lambda m: col("gd", m), lambda m: col("bd", m), out_cb)
        for m in range(KD):
            nc.sync.dma_start(out[m, :, :], outs[m][:, 0:SH].bitcast(f32))
        pz5_cm.__exit__(None, None, None)
        ph_cm.__exit__(None, None, None)

        for cm in reversed(es):
            cm.__exit__(None, None, None)
        rep_ctx.__exit__(None, None, None)

    nc.compile()
    return tap_names


def _prep_inputs(inputs):
    """Host-side sharding: returns in_maps (list of 8 dicts)."""
    x, y = inputs["x"], inputs["y"]
    colvecs = {
        "bq1": inputs["bq_m"], "bk1": inputs["bk_m"],
        "bq2": inputs["bq_c"], "bk2": inputs["bk_c"],
        "gm1": inputs["g_m"], "bm1": inputs["b_m"],
        "gm2": inputs["g_c"], "bm2": inputs["b_c"],
        "gd": inputs["g_d"], "bd": inputs["b_d"],
        "fb0": inputs["f0_b"], "fb1": inputs["f1_b"],
    }
    cols = np.empty((128, NCOL * 8 + 1), np.float32)
    for c, n in enumerate(COLS):
        cols[:, c * 8:(c + 1) * 8] = np.asarray(colvecs[n], np.float32) \
            .reshape(KD, 128).T
    cols[:, ONES_COL] = 1.0
    shared = {
        "w_q1": np.asarray(inputs["wq_m"], np.float32),
        "w_k1": np.asarray(inputs["wk_m"], np.float32),
        "w_v1": np.asarray(inputs["wv_m"], np.float32),
        "w_q2": np.asarray(inputs["wq_c"], np.float32),
        "w_k2": np.asarray(inputs["wk_c"], np.float32),
        "w_v2": np.asarray(inputs["wv_c"], np.float32),
        "w_f0": np.asarray(inputs["f0_w"], np.float32),
        "w_f1": np.asarray(inputs["f1_w"], np.float32),
        "cols": cols,
        "bv1": np.asarray(inputs["bv_m"], np.float32).reshape(1, D),
        "bv2": np.asarray(inputs["bv_c"], np.float32).reshape(1, D),
    }
    in_maps = []
    for c in range(N_CORES):
        b, h = c // 2, c % 2
        xT = np.ascontiguousarray(np.asarray(x[b], np.float32).T)  # [D, T]
        yT = np.ascontiguousarray(np.asarray(y[b], np.float32).T)
        m = dict(shared)
        m["xkv"] = xT.reshape(KD, 128, T)
        m["ykv"] = yT.reshape(KD, 128, T)
        m["xq"] = np.ascontiguousarray(
            xT[:, h * SH:(h + 1) * SH]).reshape(KD, 128, SH)
        in_maps.append(m)
    return in_maps


def kernel(**inputs):
    nc = bacc.Bacc("TRN2", target_bir_lowering=False, debug=False,
                   num_devices=N_CORES)
    build_decoder(nc, taps=False)
    in_maps = _prep_inputs(inputs)
    res = run_bass_kernel_spmd(nc, in_maps, core_ids=list(range(N_CORES)),
                               trace=False)
    out = np.empty((B, S, D), np.float32)
    for c in range(N_CORES):
        b, h = c // 2, c % 2
        o = res.results[c]["out"].reshape(D, SH)  # feature-major [d, s]
        out[b, h * SH:(h + 1) * SH, :] = o.T
    return out


# revision 9
# speedup vs baseline: 1.4968x; 1.4968x over previous
"""Trainium2 Bass kernel for nn_DecoderStack (self-attn + cross-attn +
2-layer GELU FFN, shared decoder LN), 8-core data-parallel.

Sharding: 8 cores = 4 batches x 2 query-halves. Core c handles batch b=c//2,
query half h=c%2 (1024 tokens); K/V context is the full 2048 tokens of its
batch element (inputs only; no collectives).

Math restructuring (exact, up to float32r rounding):
  * softmax is invariant to the K-bias term, so  scores.T = x_kvT @ P  with
    P = (wq @ wk.T).T @ q_in + (wk @ bq)  — a single 1024-token projection
    replaces Q-proj and the 2048-token K-proj (host precomputes wq@wk.T).
  * PV is reassociated:  U = wv.T @ G + bv*denom,  G = x_tok.T-contraction
    of E  — the 2048-token V-proj becomes a 1024-token projection of G.

Layout: activations feature-major [D, S] (D on partitions); all matmuls in
float32r (TF32-like, full PE rate); scores transposed [t, s]; softmax
denominator via ones-column matmuls; row broadcasts via GpSimd
partition_broadcast. G spills to DRAM between its two matmul phases.
"""
import sys
for _p in ("/opt/trn_rl_repo", "/root/.axon_site/_ro/trn_rl_repo"):
    if _p not in sys.path:
        sys.path.append(_p)

import numpy as np

import concourse.bass as bass
import concourse.tile as tile
from concourse import bacc, mybir
from concourse.bass_utils import run_bass_kernel_spmd

f32 = mybir.dt.float32
f32r = mybir.dt.float32r
AF = mybir.ActivationFunctionType
ALU = mybir.AluOpType

N_CORES = 8
B, S, T, D = 4, 2048, 2048, 1024
SH = S // 2          # per-core query tokens
KD = D // 128        # 8 d-tiles
TM = T // 128        # 16 t-tiles
SCALE = 1.0 / 8.0
LN_EPS = 1e-5
LN_RD = 1.0 / D

COLS = ["cp1", "cp2", "bv1", "bv2", "gm1", "bm1", "gm2", "bm2",
        "gd", "bd", "fb0", "fb1"]
NCOL = len(COLS)
ONES_COL = NCOL * 8  # last column of the packed cols input


def build_decoder(nc, taps=False, reps=0):
    """Emit the full per-core decoder program. Returns tap tensor names."""
    def din(name, shape, dt=f32r):
        return nc.dram_tensor(name, shape, dt, kind="ExternalInput").ap()

    xq = din("xq", [KD, 128, SH])          # x[b,half].T feature-major
    xkv = din("xkv", [KD, 128, T])         # x[b].T full (feature-major)
    ykv = din("ykv", [KD, 128, T])         # y[b].T full
    xtok = din("xtok", [TM, 128, D])       # x[b] token-major
    ytok = din("ytok", [TM, 128, D])       # y[b] token-major
    w = {n: din("w_" + n, [D, D]) for n in
         ["p1", "v1", "p2", "v2", "f0", "f1"]}
    cols_in = din("cols", [128, NCOL * 8 + 1], f32r)  # + ones column
    out = nc.dram_tensor("out", [KD, 128, SH], f32, kind="ExternalOutput").ap()

    tap_names = []

    with tile.TileContext(nc, pool_alloc_mode="queue") as tc:
        import contextlib
        rep_ctx = tc.For_i(0, reps, 1) if reps else contextlib.nullcontext()
        es = []

        def open_pool(name, bufs=1, space="SBUF"):
            cm = tc.tile_pool(name=name, bufs=bufs, space=space)
            pool = cm.__enter__()
            es.append(cm)
            return pool

        rep_ctx.__enter__()
        p_w = open_pool("w", bufs=2)          # weight halves [128,8,512] 16K
        p_st4 = open_pool("st4", bufs=3)      # [128,8,128] tile streams 4K
        p_stage = open_pool("stage", bufs=4)  # [128,1024] staging 4K
        p_bc = open_pool("bc", bufs=2)        # [128,1024] broadcasts 4K
        p_rows = open_pool("rows", bufs=3)    # [1,1024] rows 4K
        p_cmn = open_pool("cmn", bufs=1)      # cols 4K
        p_dram = open_pool("dram", bufs=1, space="DRAM")
        p_psm = open_pool("psm", bufs=4, space="PSUM")   # [128,512]
        p_psr = open_pool("psr", bufs=4, space="PSUM")   # [1,512]

        cols_sb = p_cmn.tile([128, NCOL * 8 + 1], f32r, name="cols_sb")
        nc.sync.dma_start(cols_sb[:], cols_in)
        ones_sb = cols_sb[:, ONES_COL:ONES_COL + 1]      # f32r ones column

        def col(name, j):
            c = COLS.index(name)
            return cols_sb[:, c * 8 + j: c * 8 + j + 1].bitcast(f32)

        def tap(name, ap_src, shape, dt_src=f32r):
            if not taps:
                return
            t = nc.dram_tensor("tap_" + name, shape, f32,
                               kind="ExternalOutput").ap()
            tap_names.append("tap_" + name)
            nc.sync.dma_start(t, ap_src.bitcast(f32) if dt_src == f32r else ap_src)

        def load_w_halves(wap):
            """Weight [D, D] as two halves [128, 8, 512] (d_out split)."""
            wr = wap.rearrange("(ko kp) d -> kp ko d", kp=128)
            halves = []
            for hf in range(2):
                t = p_w.tile([128, KD, 512], f32r, tag="w", name=f"wh{hf}")
                nc.sync.dma_start(t[:], wr[:, :, hf * 512:(hf + 1) * 512])
                halves.append(t)
            return halves

        def proj(out_write, wap, rhs_src, n_tok):
            """Feature-major projection: psum[m-tile, 512chunk] = w.T @ rhs.

            out_write(m, tch, ps): epilogue for the [128,512] PSUM tile.
            rhs_src: DRAM AP [KD, 128, n_tok] or SBUF tile [128, KD, n_tok].
            """
            wh = load_w_halves(wap)
            nch = n_tok // 512
            from_dram = rhs_src.space == bass.MemorySpace.DRAM
            pk_cm = tc.tile_pool(name="kvch", bufs=2)
            pk = pk_cm.__enter__()
            for tch in range(nch):
                sl = slice(tch * 512, (tch + 1) * 512)
                if from_dram:
                    kvc = pk.tile([128, KD, 512], f32r, tag="kv", name="kvc")
                    nc.sync.dma_start(
                        kvc[:], rhs_src[:, :, sl].rearrange("ko p s -> p ko s"))
                    rhs = lambda k: kvc[:, k, :]
                else:
                    rhs = lambda k, sl=sl: rhs_src[:, k, sl]
                for m in range(KD):
                    ps = p_psm.tile([128, 512], f32, tag="mm", name="proj_ps")
                    whf = wh[m // 4]
                    ml = m % 4
                    for k in range(KD):
                        nc.tensor.matmul(
                            ps[:], lhsT=whf[:, k, ml * 128:(ml + 1) * 128],
                            rhs=rhs(k), start=(k == 0), stop=(k == KD - 1))
                    out_write(m, tch, ps)
            pk_cm.__exit__(None, None, None)

        def layernorm(z_sb, g_cb, b_cb, out_cb):
            """LN over the feature dim (128 partitions x KD) of [128,KD,SH]."""
            ps_s = [p_psr.tile([1, 512], f32, tag="row", name=f"lns{i}")
                    for i in range(2)]
            ps_q = [p_psr.tile([1, 512], f32, tag="row", name=f"lnq{i}")
                    for i in range(2)]
            for m in range(KD):
                sq = p_stage.tile([128, 1024], f32r, tag="stage", name="lnsq")
                nc.scalar.activation(sq[:], z_sb[:, m, :], AF.Square)
                for sch in range(2):
                    sl = slice(sch * 512, (sch + 1) * 512)
                    nc.tensor.matmul(ps_s[sch][:], lhsT=ones_sb,
                                     rhs=z_sb[:, m, sl],
                                     start=(m == 0), stop=(m == KD - 1))
                    nc.tensor.matmul(ps_q[sch][:], lhsT=ones_sb,
                                     rhs=sq[:, sl],
                                     start=(m == 0), stop=(m == KD - 1))
            sum_r = p_rows.tile([1, 1024], f32, tag="row", name="sum_r")
            sq_r = p_rows.tile([1, 1024], f32, tag="row", name="sq_r")
            for sch in range(2):
                sl = slice(sch * 512, (sch + 1) * 512)
                nc.scalar.copy(sum_r[:, sl], ps_s[sch][:])
                nc.scalar.copy(sq_r[:, sl], ps_q[sch][:])
            mean = p_rows.tile([1, 1024], f32, tag="mean", bufs=1, name="mean")
            nc.vector.tensor_scalar(mean[:], sum_r[:], LN_RD, None, op0=ALU.mult)
            vp = p_rows.tile([1, 1024], f32, tag="row", name="vp")
            nc.vector.tensor_scalar(vp[:], sq_r[:], LN_RD, LN_EPS,
                                    op0=ALU.mult, op1=ALU.add)
            msq = p_rows.tile([1, 1024], f32, tag="row", name="msq")
            nc.vector.tensor_mul(msq[:], mean[:], mean[:])
            varc = p_rows.tile([1, 1024], f32, tag="row", name="varc")
            nc.vector.tensor_sub(varc[:], vp[:], msq[:])
            std = p_rows.tile([1, 1024], f32, tag="row", name="std")
            nc.scalar.activation(std[:], varc[:], AF.Sqrt)
            rstd = p_rows.tile([1, 1024], f32, tag="row", name="rstd")
            nc.vector.reciprocal(rstd[:], std[:])
            cr = p_rows.tile([1, 1024], f32, tag="row", name="cr")
            nc.vector.tensor_mul(cr[:], mean[:], rstd[:])
            rstd_bc = p_bc.tile([128, 1024], f32, tag="bc", name="rstd_bc")
            nc.gpsimd.partition_broadcast(rstd_bc[:], rstd[:])
            c_bc = p_bc.tile([128, 1024], f32, tag="bc", name="c_bc")
            nc.gpsimd.partition_broadcast(c_bc[:], cr[:])
            for m in range(KD):
                t1 = p_stage.tile([128, 1024], f32, tag="stage", name="ln_t1")
                nc.vector.tensor_mul(t1[:], z_sb[:, m, :].bitcast(f32), rstd_bc[:])
                t2 = p_stage.tile([128, 1024], f32, tag="stage", name="ln_t2")
                nc.vector.tensor_sub(t2[:], t1[:], c_bc[:])
                nc.vector.tensor_scalar(out_cb(m), t2[:], g_cb(m), b_cb(m),
                                        op0=ALU.mult, op1=ALU.add)

        def attention_block(qin_d, kvF_d, kvT_d, wP, wV, cpn, bvn,
                            gmn, bmn, xout_d, blk):
            # ---- P projection (SBUF resident): P = wP.T @ qin + cp ----
            pq_cm = tc.tile_pool(name=f"p{blk}", bufs=1)
            pq = pq_cm.__enter__()
            p_sb = pq.tile([128, KD, SH], f32r, name="p_sb")

            def pwrite(m, sch, ps):
                nc.vector.tensor_scalar(p_sb[:, m, sch * 512:(sch + 1) * 512],
                                        ps[:], col(cpn, m), None, op0=ALU.add)
            proj(pwrite, wP, qin_d, SH)
            tap(f"P{blk}", p_sb[:], [128, KD, SH])

            # ---- scores.T = kvF.T-contraction of P ; exp ; denominator ----
            pe_cm = tc.tile_pool(name=f"e{blk}", bufs=1)
            pe = pe_cm.__enter__()
            e_sb = pe.tile([128, TM, SH], f32r, name="e_sb")
            ps_d = [p_psr.tile([1, 512], f32, tag="row", name=f"dn{i}")
                    for i in range(2)]
            for tm in range(TM):
                kt = p_st4.tile([128, KD, 128], f32r, tag="st4", name="kt")
                nc.sync.dma_start(
                    kt[:], kvF_d[:, :, tm * 128:(tm + 1) * 128]
                    .rearrange("ko p t -> p ko t"))
                for sch in range(2):
                    sl = slice(sch * 512, (sch + 1) * 512)
                    ps = p_psm.tile([128, 512], f32, tag="mm", name="sc_ps")
                    for k in range(KD):
                        nc.tensor.matmul(ps[:], lhsT=kt[:, k, :],
                                         rhs=p_sb[:, k, sl],
                                         start=(k == 0), stop=(k == KD - 1))
                    nc.scalar.activation(e_sb[:, tm, sl], ps[:], AF.Exp,
                                         scale=SCALE)
                    nc.tensor.matmul(ps_d[sch][:], lhsT=ones_sb,
                                     rhs=e_sb[:, tm, sl],
                                     start=(tm == 0), stop=(tm == TM - 1))
            den_r = p_rows.tile([1, 1024], f32, tag="row", name="den_r")
            for sch in range(2):
                nc.scalar.copy(den_r[:, sch * 512:(sch + 1) * 512], ps_d[sch][:])
            tap(f"den{blk}", den_r[:], [1, 1024], f32)
            rden = p_rows.tile([1, 1024], f32, tag="row", name="rden")
            nc.vector.reciprocal(rden[:], den_r[:])
            rden_bc = p_bc.tile([128, 1024], f32, tag="bc", name="rden_bc")
            nc.gpsimd.partition_broadcast(rden_bc[:], rden[:])

            # ---- G = kvT.T-contraction of E -> DRAM [KD, 128, SH] ----
            g_spill = p_dram.tile([KD, 128, SH], f32r, tag="gspill",
                                  name="g_spill")
            for m in range(KD):
                vh = []
                for hfm in range(2):
                    vt = p_st4.tile([128, 8, 128], f32r, tag="st4", name="vh")
                    nc.sync.dma_start(
                        vt[:], kvT_d[hfm * 8:(hfm + 1) * 8, :,
                                     m * 128:(m + 1) * 128]
                        .rearrange("tm p d -> p tm d"))
                    vh.append(vt)
                psu = [p_psm.tile([128, 512], f32, tag="mm", name=f"pv{i}")
                       for i in range(2)]
                for tm in range(TM):
                    vt = vh[tm // 8][:, tm % 8, :]
                    for sch in range(2):
                        sl = slice(sch * 512, (sch + 1) * 512)
                        nc.tensor.matmul(psu[sch][:], lhsT=vt,
                                         rhs=e_sb[:, tm, sl],
                                         start=(tm == 0), stop=(tm == TM - 1))
                st = p_stage.tile([128, 1024], f32r, tag="stage", name="g_st")
                for sch in range(2):
                    nc.scalar.copy(st[:, sch * 512:(sch + 1) * 512],
                                   psu[sch][:])
                nc.sync.dma_start(g_spill[m, :, :], st[:])
            pe_cm.__exit__(None, None, None)

            # ---- U = wV.T @ G ; normalize ; +bv ; +resid -> Z ----
            pz_cm = tc.tile_pool(name=f"z{blk}", bufs=1)
            pz = pz_cm.__enter__()
            z_sb = pz.tile([128, KD, SH], f32r, name="z_sb")

            def uwrite(m, sch, ps):
                sl = slice(sch * 512, (sch + 1) * 512)
                rt = p_stage.tile([128, 1024], f32r, tag="stage", name="res_t")
                nc.sync.dma_start(rt[:, 0:512], qin_d[m, :, sl])
                t1 = p_stage.tile([128, 1024], f32, tag="stage", name="pv_t1")
                nc.vector.tensor_mul(t1[:, 0:512], ps[:], rden_bc[:, sl])
                t2 = p_stage.tile([128, 1024], f32, tag="stage", name="pv_t2")
                nc.vector.tensor_add(t2[:, 0:512], t1[:, 0:512],
                                     rt[:, 0:512].bitcast(f32))
                nc.vector.tensor_scalar(z_sb[:, m, sl], t2[:, 0:512],
                                        col(bvn, m), None, op0=ALU.add)
            proj(uwrite, wV, g_spill, SH)
            tap(f"Z1_{blk}", z_sb[:], [128, KD, SH])

            # ---- LN_m (in-place) ; + resid (in-place) ; LN_d -> xout ----
            layernorm(z_sb, lambda m: col(gmn, m), lambda m: col(bmn, m),
                      lambda m: z_sb[:, m, :])
            for m in range(KD):
                for sch in range(2):
                    sl = slice(sch * 512, (sch + 1) * 512)
                    rt = p_stage.tile([128, 1024], f32r, tag="stage",
                                      name="res2_t")
                    nc.sync.dma_start(rt[:, 0:512], qin_d[m, :, sl])
                    nc.vector.tensor_add(z_sb[:, m, sl], z_sb[:, m, sl],
                                         rt[:, 0:512])
            sts = {}

            def xcb(m):
                st = p_stage.tile([128, 1024], f32r, tag="stage", name="xo_st")
                sts[m] = st
                return st[:, 0:SH]
            layernorm(z_sb, lambda m: col("gd", m), lambda m: col("bd", m), xcb)
            for m in range(KD):
                nc.sync.dma_start(xout_d[m, :, :], sts[m][:, 0:SH])
            pz_cm.__exit__(None, None, None)
            pq_cm.__exit__(None, None, None)

        # ================= decoder =================
        x1_d = p_dram.tile([KD, 128, SH], f32r, tag="x1", name="x1_d")
        attention_block(xq, xkv, xtok, w["p1"], w["v1"], "cp1", "bv1",
                        "gm1", "bm1", x1_d, 1)
        x2_d = p_dram.tile([KD, 128, SH], f32r, tag="x2", name="x2_d")
        attention_block(x1_d, ykv, ytok, w["p2"], w["v2"], "cp2", "bv2",
                        "gm2", "bm2", x2_d, 2)

        # ================= FFN =================
        ph_cm = tc.tile_pool(name="h1p", bufs=1)
        ph = ph_cm.__enter__()
        h1 = ph.tile([128, KD, SH], f32r, name="h1")

        def h1w(m, sch, ps):
            nc.scalar.activation(h1[:, m, sch * 512:(sch + 1) * 512], ps[:],
                                 AF.Gelu, bias=col("fb0", m))
        proj(h1w, w["f0"], x2_d, SH)

        pz5_cm = tc.tile_pool(name="z5p", bufs=1)
        pz5 = pz5_cm.__enter__()
        z5 = pz5.tile([128, KD, SH], f32r, name="z5")

        def h2w(m, sch, ps):
            sl = slice(sch * 512, (sch + 1) * 512)
            t1 = p_stage.tile([128, 1024], f32, tag="stage", name="h2_t")
            nc.scalar.activation(t1[:, 0:512], ps[:], AF.Gelu,
                                 bias=col("fb1", m))
            rt = p_stage.tile([128, 1024], f32r, tag="stage", name="resf_t")
            nc.sync.dma_start(rt[:, 0:512], x2_d[m, :, sl])
            nc.vector.tensor_add(z5[:, m, sl], t1[:, 0:512],
                                 rt[:, 0:512].bitcast(f32))
        proj(h2w, w["f1"], h1, SH)

        outs = {}

        def out_cb(m):
            st = p_stage.tile([128, 1024], f32r, tag="stage", name="out_st")
            outs[m] = st
            return st[:, 0:SH]
        layernorm(z5, lambda m: col("gd", m), lambda m: col("bd", m), out_cb)
        for m in range(KD):
            nc.sync.dma_start(out[m, :, :], outs[m][:, 0:SH].bitcast(f32))
        pz5_cm.__exit__(None, None, None)
        ph_cm.__exit__(None, None, None)

        for cm in reversed(es):
            cm.__exit__(None, None, None)
        rep_ctx.__exit__(None, None, None)

    nc.compile()
    return tap_names


def _prep_inputs(inputs):
    """Host-side sharding + weight folding: returns in_maps (8 dicts)."""
    f64 = lambda k: np.asarray(inputs[k], np.float64)
    x, y = inputs["x"], inputs["y"]
    # folded attention weights: P = (wq@wk.T).T @ qin + wk@bq
    wp1 = (f64("wq_m") @ f64("wk_m").T).astype(np.float32)
    cp1 = (f64("wk_m") @ f64("bq_m")).astype(np.float32)
    wp2 = (f64("wq_c") @ f64("wk_c").T).astype(np.float32)
    cp2 = (f64("wk_c") @ f64("bq_c")).astype(np.float32)
    colvecs = {
        "cp1": cp1, "cp2": cp2,
        "bv1": inputs["bv_m"], "bv2": inputs["bv_c"],
        "gm1": inputs["g_m"], "bm1": inputs["b_m"],
        "gm2": inputs["g_c"], "bm2": inputs["b_c"],
        "gd": inputs["g_d"], "bd": inputs["b_d"],
        "fb0": inputs["f0_b"], "fb1": inputs["f1_b"],
    }
    cols = np.empty((128, NCOL * 8 + 1), np.float32)
    for c, n in enumerate(COLS):
        cols[:, c * 8:(c + 1) * 8] = np.asarray(colvecs[n], np.float32) \
            .reshape(KD, 128).T
    cols[:, ONES_COL] = 1.0
    shared = {
        "w_p1": wp1, "w_p2": wp2,
        "w_v1": np.asarray(inputs["wv_m"], np.float32),
        "w_v2": np.asarray(inputs["wv_c"], np.float32),
        "w_f0": np.asarray(inputs["f0_w"], np.float32),
        "w_f1": np.asarray(inputs["f1_w"], np.float32),
        "cols": cols,
    }
    in_maps = []
    for c in range(N_CORES):
        b, h = c // 2, c % 2
        xb = np.asarray(x[b], np.float32)
        yb = np.asarray(y[b], np.float32)
        xT = np.ascontiguousarray(xb.T)  # [D, T]
        yT = np.ascontiguousarray(yb.T)
        m = dict(shared)
        m["xkv"] = xT.reshape(KD, 128, T)
        m["ykv"] = yT.reshape(KD, 128, T)
        m["xtok"] = np.ascontiguousarray(xb).reshape(TM, 128, D)
        m["ytok"] = np.ascontiguousarray(yb).reshape(TM, 128, D)
        m["xq"] = np.ascontiguousarray(
            xT[:, h * SH:(h + 1) * SH]).reshape(KD, 128, SH)
        in_maps.append(m)
    return in_maps


def kernel(**inputs):
    nc = bacc.Bacc("TRN2", target_bir_lowering=False, debug=False,
                   num_devices=N_CORES)
    build_decoder(nc, taps=False)
    in_maps = _prep_inputs(inputs)
    res = run_bass_kernel_spmd(nc, in_maps, core_ids=list(range(N_CORES)),
                               trace=False)
    out = np.empty((B, S, D), np.float32)
    for c in range(N_CORES):
        b, h = c // 2, c % 2
        o = res.results[c]["out"].reshape(D, SH)  # feature-major [d, s]
        out[b, h * SH:(h + 1) * SH, :] = o.T
    return out
